# revision 1
# baseline (speedup 1.0000x reference)
"""Trainium2 Bass kernel for fused attention + LayerNorm + projection.

Computation (per reference):
    q = input1 @ Wq + bq                       [8192, 32]
    k = input2 @ Wk + bk                       [8192, 32]
    v = input2 @ Wv + bv                       [8192, 32]
    P = softmax(q @ k.T, axis=-1)              [8192, 8192]
    fused = P @ v                              [8192, 32]
    out = LayerNorm(fused) * gamma + beta @ Wo + bo   [8192, 128]

Sharding: data-parallel over rows of input1 (1024 rows per core, 8 cores);
input2 and weights replicated.

Key algebraic simplifications used on-device:
  - softmax normalization (and max-subtraction) is skipped: LayerNorm is
    invariant to a positive per-row scale, so exp(s) @ v is normalized for
    free by LN (eps term differs by ~1e-3 relative — validated vs reference).
  - gamma is folded into Wo (diag(gamma) @ Wo) and beta/bo folded into an
    extra contraction row via an augmented ones-row, both on the host.

Dataflow per core:
  - in2 chunks [128,128] are PE-transposed (fp32); kT = Wk.T @ in2T computed
    with 4x column-tiled fp32 matmuls so kT chunk c lands at PSUM partitions
    32*(c%4) (the "banded" kstack layout); v = in2T.T @ Wv natural [n, 32].
  - qT is computed replicated into all 4 partition bands (column tiling).
  - scoresT[n, m] chunks via 4x row-tiled float32r matmuls (K=32): float32r
    streams at 1 cycle/column (fp32 is 4x slower); column tiling is
    incompatible with float32r on this compiler, row tiling is fine. All
    float32r matmul inputs must be produced by a rounding instruction
    (DVE/ACT output into a float32r tile), not DMA.
  - exp runs on ACT straight out of PSUM (the bottleneck engine, ~64us/core:
    8.4M elements at 1 elem/lane/cycle @ 1.2 GHz).
  - AV accumulates fusedT = v.T @ P.T in bf16 (column-tiled, 4 partition
    bands in one PSUM bank, summed at the end on DVE). bf16 error averages
    out over the ~1000s of softmax terms (<0.1% contribution).
  - LayerNorm stats via bn_stats/bn_aggr; rstd = exp(-0.5*ln(var+eps)) so
    the ACT table set stays in the ln/exp family (no table switch).
"""

import os
import sys

import numpy as np

N1 = 8192
N2 = 8192
DIN = 128
D = 32
DOUT = 128
NCORES = 8
MSH = N1 // NCORES          # rows per core
NCH = N2 // 128             # 64 in2 chunks
NG = NCH // 4               # 16 groups of 4 chunks
NMB = MSH // 512            # 2 m-passes of 512 rows
LN_EPS = 1e-5

_CACHE = {}


def _import_concourse():
    try:
        import concourse.bass  # noqa: F401
    except ImportError:
        for p in ("/opt/trn_rl_repo", os.path.expanduser("~/.axon_site/_ro/trn_rl_repo")):
            if os.path.isdir(p) and p not in sys.path:
                sys.path.insert(0, p)


def build(reps=1):
    """Build (and cache) the compiled single-core SPMD Bass program.

    reps > 1 repeats the whole computation serially (for wall-clock slope
    timing); the output is rewritten identically each rep.
    """
    key = ("nc", reps)
    if key in _CACHE:
        return _CACHE[key]
    _import_concourse()
    import concourse.bacc as bacc
    import concourse.tile as tile
    from concourse import mybir

    f32 = mybir.dt.float32
    AF = mybir.ActivationFunctionType
    OP = mybir.AluOpType

    f32r = mybir.dt.float32r
    bf16 = mybir.dt.bfloat16

    nc = bacc.Bacc(None, target_bir_lowering=False, debug=False)

    x1 = nc.dram_tensor("x1", [MSH, DIN], f32, kind="ExternalInput")
    x2 = nc.dram_tensor("x2", [N2, DIN], f32, kind="ExternalInput")
    wq_d = nc.dram_tensor("wq", [DIN, D], f32, kind="ExternalInput")
    wk_d = nc.dram_tensor("wk", [DIN, D], f32, kind="ExternalInput")
    wv_d = nc.dram_tensor("wv", [DIN, D], f32, kind="ExternalInput")
    bq4_d = nc.dram_tensor("bq4", [128, 1], f32, kind="ExternalInput")
    bk4_d = nc.dram_tensor("bk4", [128, 1], f32, kind="ExternalInput")
    bvb_d = nc.dram_tensor("bvb", [128, D], f32, kind="ExternalInput")
    woa_d = nc.dram_tensor("woa", [D + 1, DOUT], f32, kind="ExternalInput")
    id_d = nc.dram_tensor("ident", [128, 128], f32, kind="ExternalInput")
    eps_d = nc.dram_tensor("epsc", [128, 1], f32, kind="ExternalInput")
    out_d = nc.dram_tensor("out", [MSH, DOUT], f32, kind="ExternalOutput")

    from contextlib import ExitStack

    with tile.TileContext(nc) as tc, ExitStack() as outer:
        consts = outer.enter_context(tc.tile_pool(name="consts", bufs=1))
        persist = outer.enter_context(tc.tile_pool(name="persist", bufs=1))

        ident = consts.tile([128, 128], f32)
        nc.sync.dma_start(out=ident, in_=id_d[:])
        wq = consts.tile([DIN, D], f32)
        nc.sync.dma_start(out=wq, in_=wq_d[:])
        wk = consts.tile([DIN, D], f32)
        nc.sync.dma_start(out=wk, in_=wk_d[:])
        wv = consts.tile([DIN, D], f32)
        nc.sync.dma_start(out=wv, in_=wv_d[:])
        bq4 = consts.tile([128, 1], f32)
        nc.sync.dma_start(out=bq4, in_=bq4_d[:])
        bk4 = consts.tile([128, 1], f32)
        nc.sync.dma_start(out=bk4, in_=bk4_d[:])
        bvb = consts.tile([128, D], f32)
        nc.sync.dma_start(out=bvb, in_=bvb_d[:])
        woa = consts.tile([D + 1, DOUT], f32)
        nc.sync.dma_start(out=woa, in_=woa_d[:])
        epsc = consts.tile([128, 1], f32)
        nc.sync.dma_start(out=epsc, in_=eps_d[:])
        wv_r = consts.tile([DIN, D], f32r)
        nc.vector.tensor_copy(wv_r, wv)
        woa_r = consts.tile([D + 1, DOUT], f32r)
        nc.vector.tensor_copy(woa_r, woa)

        # Pull the exp table load (~2.7us) into the initial DMA window.
        warm = consts.tile([1, 8], f32)
        nc.scalar.activation(warm, ident[0:1, 0:8], AF.Exp)

        kstack = persist.tile([128, NG * 128], f32r)    # kT chunk c: [32*(c%4):+32, 128*(c//4):+128]
        vstack = persist.tile([128, NCH * D], bf16)     # v chunk c: [:, 32*c:+32]
        qt_rep = persist.tile([128, MSH], f32r)         # qT replicated in 4 bands
        x1t_all = persist.tile([128, MSH], f32)        # input1 shard transposed
        fusedT = persist.tile([128, MSH], f32)         # rows 0:32 = v.T @ P.T, rows 32:128 = 0
        for p0 in range(D, 128, 32):
            nc.vector.memset(fusedT[p0:p0 + 32, :], 0.0)

        for _rep in range(reps):
          with (
            tc.tile_pool(name="qload", bufs=1) as qload,
            tc.tile_pool(name="x2load", bufs=3) as x2load,
            tc.tile_pool(name="i2t_sb", bufs=6) as i2t_sbp,
            tc.tile_pool(name="pp_ps", bufs=2, space="PSUM") as pp_ps,
            tc.tile_pool(name="sc_ps", bufs=2, space="PSUM") as sc_ps,
            tc.tile_pool(name="av_ps", bufs=2, space="PSUM") as av_ps,
            tc.tile_pool(name="pt", bufs=4) as ptp,
            tc.tile_pool(name="tmp32", bufs=2) as tmp32p,
        ):
            # ---- q prep: x1 -> x1T -> qT replicated into 4 bands (+bq) ----
            x1_sb = qload.tile([128, MSH // 128, 128], f32)
            nc.sync.dma_start(
                out=x1_sb, in_=x1[:].rearrange("(t p) d -> p t d", p=128)
            )
            for t in range(MSH // 128):
                tps = sc_ps.tile([128, 128], f32, tag="sc")
                nc.tensor.transpose(tps, x1_sb[:, t, :], ident)
                nc.vector.tensor_copy(x1t_all[:, t * 128:(t + 1) * 128], tps)
            for t2 in range(MSH // 256):
                qps = sc_ps.tile([128, 256], f32, tag="sc")
                for j in range(4):
                    nc.tensor.matmul(
                        qps[32 * j:32 * (j + 1), :],
                        lhsT=wq,
                        rhs=x1t_all[:, t2 * 256:(t2 + 1) * 256],
                        start=True,
                        stop=True,
                        tile_position=(0, 32 * j),
                    )
                nc.vector.tensor_scalar_add(
                    qt_rep[:, t2 * 256:(t2 + 1) * 256], qps, bq4
                )

            # ---- streaming: in2 prep (group g) + attention pass 0 (group g) ----
            av_acc = [None, None]

            def prep_group(g):
                x2_sb = x2load.tile([128, 4, 128], f32, tag="x2")
                nc.sync.dma_start(
                    out=x2_sb,
                    in_=x2[g * 512:(g + 1) * 512, :].rearrange(
                        "(p t) d -> p t d", p=128
                    ),
                )
                i2t = []
                for j in range(4):
                    tps = pp_ps.tile([128, 128], f32, tag="pp")
                    nc.tensor.transpose(tps, x2_sb[:, j, :], ident)
                    sb = i2t_sbp.tile([128, 128], f32r, tag="i2t")
                    nc.vector.tensor_copy(sb, tps)
                    i2t.append(sb)
                for j in range(4):
                    c = 4 * g + j
                    vps = pp_ps.tile([128, D], f32, tag="pp")
                    nc.tensor.matmul(vps, lhsT=i2t[j], rhs=wv_r, start=True, stop=True)
                    nc.vector.tensor_add(vstack[:, D * c:D * (c + 1)], vps, bvb)
                kps = pp_ps.tile([128, 128], f32, tag="pp")
                for j in range(4):
                    nc.tensor.matmul(
                        kps[32 * j:32 * (j + 1), :],
                        lhsT=wk,
                        rhs=i2t[j].bitcast(f32),
                        start=True,
                        stop=True,
                        tile_position=(0, 32 * j),
                    )
                nc.vector.tensor_scalar_add(
                    kstack[:, g * 128:(g + 1) * 128], kps, bk4
                )

            def attn_group(p, g):
                m0 = p * 512
                for h in range(2):
                    sps = sc_ps.tile([128, 1024], f32, tag="sc")
                    for ci in range(2):
                        c = 4 * g + 2 * h + ci
                        j = c % 4
                        nc.tensor.matmul(
                            sps[:, 512 * ci:512 * (ci + 1)],
                            lhsT=kstack[32 * j:32 * (j + 1), g * 128:(g + 1) * 128],
                            rhs=qt_rep[32 * j:32 * (j + 1), m0:m0 + 512],
                            start=True,
                            stop=True,
                            tile_position=(32 * j, 0),
                        )
                    pt = ptp.tile([128, 1024], bf16, tag="pt")
                    nc.scalar.activation(pt, sps, AF.Exp)
                    for ci in range(2):
                        c = 4 * g + 2 * h + ci
                        j = c % 4
                        nc.tensor.matmul(
                            av_acc[p][32 * j:32 * (j + 1), :],
                            lhsT=vstack[:, D * c:D * (c + 1)],
                            rhs=pt[:, 512 * ci:512 * (ci + 1)],
                            start=(g == 0),
                            stop=(g == NG - 1),
                            tile_position=(0, 32 * j),
                            skip_group_check=True,
                        )

            def band_reduce(p):
                t1 = tmp32p.tile([D, 512], f32, tag="t1")
                nc.vector.tensor_copy(t1, av_acc[p][0:32, :])
                t2 = tmp32p.tile([D, 512], f32, tag="t2")
                nc.vector.tensor_add(t2, t1, av_acc[p][32:64, :])
                nc.vector.tensor_add(t1, t2, av_acc[p][64:96, :])
                nc.vector.tensor_add(
                    fusedT[0:D, p * 512:(p + 1) * 512], t1, av_acc[p][96:128, :]
                )

            av_acc[0] = av_ps.tile([128, 512], f32, tag="av", name="av0")
            av_acc[1] = av_ps.tile([128, 512], f32, tag="av", name="av1")
            for g in range(NG):
                prep_group(g)
                attn_group(0, g)
                attn_group(1, g)
            band_reduce(0)
            band_reduce(1)

          # ---- phase B: LayerNorm + output projection, batched over 8 blocks ----
          with (
            tc.tile_pool(name="pb_ps", bufs=2, space="PSUM") as pb_ps,
            tc.tile_pool(name="fb", bufs=2) as fbp,
            tc.tile_pool(name="cent", bufs=8) as centp,
            tc.tile_pool(name="stat", bufs=2) as statp,
            tc.tile_pool(name="lnagg", bufs=1) as lnagg,
            tc.tile_pool(name="naug", bufs=2) as naugp,
            tc.tile_pool(name="outsb", bufs=2) as outsbp,
        ):
            mv_all = lnagg.tile([128, MSH // 128, 2], f32)
            cents = []
            for b in range(MSH // 128):
                fps = pb_ps.tile([128, 128], f32, tag="pb")
                nc.tensor.transpose(fps, fusedT[:, b * 128:(b + 1) * 128], ident)
                fsb = fbp.tile([128, D], f32, tag="f")
                nc.vector.tensor_copy(fsb, fps[:, 0:D])
                st = statp.tile([128, 6], f32, tag="st")
                nc.vector.bn_stats(out=st, in_=fsb)
                nc.vector.bn_aggr(out=mv_all[:, b, :], in_=st)
                cent = centp.tile([128, 128], f32, tag="c")
                nc.vector.memset(cent[:, D:128], 0.0)
                nc.vector.tensor_scalar(
                    cent[:, 0:D], fsb, mv_all[:, b, 0:1], None, op0=OP.subtract
                )
                cents.append(cent)
            lnv = lnagg.tile([128, MSH // 128], f32)
            nc.scalar.activation(lnv, mv_all[:, :, 1], AF.Ln, bias=epsc)
            rstd = lnagg.tile([128, MSH // 128], f32)
            nc.scalar.activation(rstd, lnv, AF.Exp, scale=-0.5)
            for b in range(MSH // 128):
                nc.vector.tensor_scalar_mul(
                    cents[b][:, 0:D], cents[b][:, 0:D], rstd[:, b:b + 1]
                )
                nps = pb_ps.tile([128, 128], f32, tag="pb")
                nc.tensor.transpose(nps, cents[b], ident)
                na = naugp.tile([D + 1, 128], f32r, tag="na")
                nc.vector.tensor_copy(na[0:D, :], nps[0:D, :])
                nc.vector.tensor_scalar(
                    na[D:D + 1, :], ident[0:1, 0:128], 0.0, 1.0,
                    op0=OP.mult, op1=OP.add,
                )
                ops = pb_ps.tile([128, 128], f32, tag="pb")
                nc.tensor.matmul(ops, lhsT=na, rhs=woa_r, start=True, stop=True)
                osb = outsbp.tile([128, DOUT], f32, tag="o")
                nc.scalar.copy(osb, ops)
                nc.sync.dma_start(out=out_d[b * 128:(b + 1) * 128, :], in_=osb)

    nc.compile()
    _CACHE[key] = nc
    return nc


def host_inputs(input1, input2, Wq, bq, Wk, bk, Wv, bv, gamma, beta, Wo, bo):
    """Per-core input maps (host-side weight folding)."""
    f32 = np.float32
    input1 = np.ascontiguousarray(np.asarray(input1, f32))
    input2 = np.ascontiguousarray(np.asarray(input2, f32))
    woa = np.concatenate(
        [np.asarray(gamma, f32)[:, None] * np.asarray(Wo, f32),
         (np.asarray(beta, f32) @ np.asarray(Wo, f32) + np.asarray(bo, f32))[None, :]],
        axis=0,
    ).astype(f32)
    common = {
        "x2": input2,
        "wq": np.ascontiguousarray(np.asarray(Wq, f32)),
        "wk": np.ascontiguousarray(np.asarray(Wk, f32)),
        "wv": np.ascontiguousarray(np.asarray(Wv, f32)),
        "bq4": np.tile(np.asarray(bq, f32), 4)[:, None].copy(),
        "bk4": np.tile(np.asarray(bk, f32), 4)[:, None].copy(),
        "bvb": np.broadcast_to(np.asarray(bv, f32), (128, D)).copy(),
        "woa": woa,
        "ident": np.eye(128, dtype=f32),
        "epsc": np.full((128, 1), LN_EPS, f32),
    }
    return [
        dict(common, x1=input1[c * MSH:(c + 1) * MSH]) for c in range(NCORES)
    ]


def kernel(input1, input2, Wq, bq, Wk, bk, Wv, bv, gamma, beta, Wo, bo):
    _import_concourse()
    from concourse.bass_utils import run_bass_kernel_spmd

    nc = build()
    in_maps = host_inputs(
        input1, input2, Wq, bq, Wk, bk, Wv, bv, gamma, beta, Wo, bo
    )
    res = run_bass_kernel_spmd(nc, in_maps, list(range(NCORES)))
    return np.concatenate(
        [np.asarray(res.results[c]["out"]) for c in range(NCORES)], axis=0
    ).astype(np.float32)



# revision 3
# speedup vs baseline: 1.0935x; 1.0935x over previous
"""Trainium2 Bass kernel for fused attention + LayerNorm + projection.

Computation (per reference):
    q = input1 @ Wq + bq                       [8192, 32]
    k = input2 @ Wk + bk                       [8192, 32]
    v = input2 @ Wv + bv                       [8192, 32]
    P = softmax(q @ k.T, axis=-1)              [8192, 8192]
    fused = P @ v                              [8192, 32]
    out = LayerNorm(fused) * gamma + beta @ Wo + bo   [8192, 128]

Sharding: data-parallel over rows of input1 (1024 rows per core, 8 cores);
input2 and weights replicated.

Algebraic simplifications (validated vs reference, rel err ~6e-3):
  - softmax normalization (and max-subtraction) skipped: LayerNorm is
    invariant to a positive per-row scale, so exp(s) @ v is normalized for
    free by LN.
  - gamma folded into Wo (diag(gamma) @ Wo), beta/bo folded into an extra
    contraction row via an augmented ones-row, on the host.

Dataflow per core (ACT-exp is the roofline: 8.4M exps at 1 elem/lane/cyc
@1.2GHz ~= 64us; everything else is arranged to overlap under it):
  - x2 is cast to bf16 on host and DMA'd HBM->SBUF through the XBAR
    transpose unit (16x128-tile transposer, ~14ns/tile): x2T [128d, 8192n]
    lands directly with the contraction dim on partitions - zero PE
    transposes for the k/v projections.
  - k: per group of 4 chunks, one 4-way column-tiled matmul quad
    (lhsT=Wk bf16 stationary in 4 col-bands, rhs=x2T chunk) -> kT banded
    into kstack f32r (chunk c at partitions 32*(c%4)).
  - v: lhsT=x2T chunk (stationary), rhs=Wv -> v natural [n,32] -> vstack
    bf16, 4 chunks per PSUM bank, one bias-add per group.
  - q: x1 stays f32 for precision; PE-transposed, then 4-way col-tiled
    matmul replicates qT into all 4 partition bands of qt_rep f32r.
  - scores: per (pass, group) ONE 4-way row-tiled f32r matmul quad
    (tile_position=(32j,0), concurrent on the PE sub-arrays) writes 4
    PSUM banks = scoresT for 4 chunks x 512 m.
  - exp on ACT straight out of PSUM, 2x [128,1024] bf16 -> pt.
  - AV: 4-way col-tiled bf16 quad accumulates fusedT into one PSUM bank
    across all 16 groups (start on g==0, stop on g==15).
  - m is processed in 2 sequential passes of 512 rows; pass 0's LayerNorm
    + output projection overlaps pass 1's attention stream.
  - PSUM: 3x [128,1024] score tiles (6 banks, quad double-buffering) +
    1 AV bank + 1 prep/LN bank = 8.
  - LayerNorm: bn_stats/bn_aggr; rstd = exp(-0.5*ln(var+eps)) keeps the
    ACT table in the ln/exp family (no table switch).
"""

import os
import sys

import numpy as np

N1 = 8192
N2 = 8192
DIN = 128
D = 32
DOUT = 128
NCORES = 8
MSH = N1 // NCORES          # rows per core
NCH = N2 // 128             # 64 in2 chunks
NG = NCH // 4               # 16 groups of 4 chunks
NP = MSH // 512             # 2 m-passes of 512 rows
LN_EPS = 1e-5

_CACHE = {}


def _import_concourse():
    try:
        import concourse.bass  # noqa: F401
    except ImportError:
        for p in ("/opt/trn_rl_repo", os.path.expanduser("~/.axon_site/_ro/trn_rl_repo")):
            if os.path.isdir(p) and p not in sys.path:
                sys.path.insert(0, p)


def build():
    """Build (and cache) the compiled single-core SPMD Bass program."""
    if "nc" in _CACHE:
        return _CACHE["nc"]
    _import_concourse()
    import concourse.bacc as bacc
    import concourse.tile as tile
    from concourse import mybir

    f32 = mybir.dt.float32
    f32r = mybir.dt.float32r
    bf16 = mybir.dt.bfloat16
    AF = mybir.ActivationFunctionType
    OP = mybir.AluOpType

    nc = bacc.Bacc(None, target_bir_lowering=False, debug=False)

    x1 = nc.dram_tensor("x1", [MSH, DIN], f32, kind="ExternalInput")
    x2b = nc.dram_tensor("x2b", [N2, DIN], bf16, kind="ExternalInput")
    wq_d = nc.dram_tensor("wq", [DIN, D], f32, kind="ExternalInput")
    wkb_d = nc.dram_tensor("wkb", [DIN, D], bf16, kind="ExternalInput")
    wvb_d = nc.dram_tensor("wvb", [DIN, D], bf16, kind="ExternalInput")
    bq4_d = nc.dram_tensor("bq4", [128, 1], f32, kind="ExternalInput")
    bk4_d = nc.dram_tensor("bk4", [128, 1], f32, kind="ExternalInput")
    bvb4_d = nc.dram_tensor("bvb4", [128, 128], f32, kind="ExternalInput")
    woa_d = nc.dram_tensor("woa", [D + 1, DOUT], f32, kind="ExternalInput")
    id_d = nc.dram_tensor("ident", [128, 128], f32, kind="ExternalInput")
    eps_d = nc.dram_tensor("epsc", [128, 1], f32, kind="ExternalInput")
    out_d = nc.dram_tensor("out", [MSH, DOUT], f32, kind="ExternalOutput")

    from contextlib import ExitStack

    with tile.TileContext(nc) as tc, ExitStack() as outer:
        consts = outer.enter_context(tc.tile_pool(name="consts", bufs=1))
        persist = outer.enter_context(tc.tile_pool(name="persist", bufs=1))

        ident = consts.tile([128, 128], f32)
        nc.sync.dma_start(out=ident, in_=id_d[:])
        wq = consts.tile([DIN, D], f32)
        nc.sync.dma_start(out=wq, in_=wq_d[:])
        wk = consts.tile([DIN, D], bf16)
        nc.sync.dma_start(out=wk, in_=wkb_d[:])
        wv = consts.tile([DIN, D], bf16)
        nc.sync.dma_start(out=wv, in_=wvb_d[:])
        bq4 = consts.tile([128, 1], f32)
        nc.sync.dma_start(out=bq4, in_=bq4_d[:])
        bk4 = consts.tile([128, 1], f32)
        nc.sync.dma_start(out=bk4, in_=bk4_d[:])
        bvb4 = consts.tile([128, 128], f32)
        nc.sync.dma_start(out=bvb4, in_=bvb4_d[:])
        woa = consts.tile([D + 1, DOUT], f32)
        nc.sync.dma_start(out=woa, in_=woa_d[:])
        epsc = consts.tile([128, 1], f32)
        nc.sync.dma_start(out=epsc, in_=eps_d[:])
        woa_r = consts.tile([D + 1, DOUT], f32r)
        nc.vector.tensor_copy(woa_r, woa)

        # Pull the exp table load (~2.7us) into the initial DMA window.
        warm = consts.tile([1, 8], f32)
        nc.scalar.activation(warm, ident[0:1, 0:8], AF.Exp)

        x2t = persist.tile([128, N2], bf16)             # x2 transposed (d on partitions)
        kstack = persist.tile([128, NG * 128], f32r)    # kT chunk c: [32*(c%4):+32, 128*(c//4):+128]
        vstack = persist.tile([128, NCH * D], bf16)     # v chunk c: [:, 32*c:+32]
        qt_rep = persist.tile([128, MSH], f32r)         # qT replicated in 4 bands
        x1t_all = persist.tile([128, MSH], f32)         # input1 shard transposed
        fusedT = persist.tile([D, MSH], f32)            # v.T @ P.T after band reduce
        # Augmented-LN lhsT tiles: rows 0:D rewritten per block, row D = ones.
        na_tiles = [persist.tile([D + 1, 128], f32r, name=f"na{i}") for i in range(2)]

        with (
            tc.tile_pool(name="sc_ps", bufs=3, space="PSUM") as sc_ps,
            tc.tile_pool(name="av_ps", bufs=1, space="PSUM") as av_ps,
            tc.tile_pool(name="pp_ps", bufs=1, space="PSUM") as pp_ps,
            tc.tile_pool(name="x1load", bufs=1) as x1load,
            tc.tile_pool(name="pt", bufs=4) as ptp,
            tc.tile_pool(name="tmp32", bufs=2) as tmp32p,
            tc.tile_pool(name="fb", bufs=4) as fbp,
            tc.tile_pool(name="cent", bufs=8) as centp,
            tc.tile_pool(name="stat", bufs=2) as statp,
            tc.tile_pool(name="lnagg", bufs=2) as lnagg,
            tc.tile_pool(name="outsb", bufs=2) as outsbp,
        ):
            for i in range(2):
                nc.vector.tensor_scalar(
                    na_tiles[i][D:D + 1, :], ident[0:1, 0:128], 0.0, 1.0,
                    op0=OP.mult, op1=OP.add,
                )

            # ---- input DMAs ----
            for g in range(NG):
                nc.sync.dma_start(
                    out=x2t[:, g * 512:(g + 1) * 512],
                    in_=x2b[g * 512:(g + 1) * 512, :],
                    transpose=True,
                )
            x1_sb = x1load.tile([128, MSH // 128, 128], f32)
            nc.sync.dma_start(
                out=x1_sb, in_=x1[:].rearrange("(t p) d -> p t d", p=128)
            )

            # ---- q prep (through sc tiles; pp stays free for k/v prep) ----
            tsp = sc_ps.tile([128, 1024], f32, tag="sc")
            for t in range(MSH // 128):
                nc.tensor.transpose(
                    tsp[:, (t % 8) * 128:(t % 8) * 128 + 128], x1_sb[:, t, :], ident
                )
                nc.vector.tensor_copy(
                    x1t_all[:, t * 128:(t + 1) * 128],
                    tsp[:, (t % 8) * 128:(t % 8) * 128 + 128],
                )
            qsp = sc_ps.tile([128, 1024], f32, tag="sc")
            for t2 in range(MSH // 512):
                for j in range(4):
                    nc.tensor.matmul(
                        qsp[32 * j:32 * (j + 1), t2 * 512:(t2 + 1) * 512],
                        lhsT=wq,
                        rhs=x1t_all[:, t2 * 512:(t2 + 1) * 512],
                        start=True,
                        stop=True,
                        tile_position=(0, 32 * j),
                    )
                nc.vector.tensor_scalar_add(
                    qt_rep[:, t2 * 512:(t2 + 1) * 512],
                    qsp[:, t2 * 512:(t2 + 1) * 512],
                    bq4,
                )

            # ---- k/v prep for one group of 4 chunks ----
            def prep_group(g):
                pp = pp_ps.tile([128, 512], f32, tag="pp")
                for j in range(4):
                    nc.tensor.matmul(
                        pp[32 * j:32 * (j + 1), 0:128],
                        lhsT=wk,
                        rhs=x2t[:, (4 * g + j) * 128:(4 * g + j + 1) * 128],
                        start=True,
                        stop=True,
                        tile_position=(0, 32 * j),
                    )
                nc.vector.tensor_scalar_add(
                    kstack[:, g * 128:(g + 1) * 128], pp[:, 0:128], bk4
                )
                for j in range(4):
                    c = 4 * g + j
                    nc.tensor.matmul(
                        pp[:, 128 + 32 * j:128 + 32 * (j + 1)],
                        lhsT=x2t[:, c * 128:(c + 1) * 128],
                        rhs=wv,
                        start=True,
                        stop=True,
                    )
                nc.vector.tensor_add(
                    vstack[:, g * 128:(g + 1) * 128], pp[:, 128:256], bvb4
                )

            # ---- one attention group: 4-way score quad, 2 exps, 4-way AV quad ----
            def attn_group(p, g, av_acc):
                m0 = p * 512
                ta = sc_ps.tile([128, 1024], f32, tag="sc")
                tb = sc_ps.tile([128, 1024], f32, tag="sc")
                for j in range(4):
                    t_ = ta if j < 2 else tb
                    c0 = 512 * (j % 2)
                    nc.tensor.matmul(
                        t_[:, c0:c0 + 512],
                        lhsT=kstack[32 * j:32 * (j + 1), g * 128:(g + 1) * 128],
                        rhs=qt_rep[32 * j:32 * (j + 1), m0:m0 + 512],
                        start=True,
                        stop=True,
                        tile_position=(32 * j, 0),
                    )
                pta = ptp.tile([128, 1024], bf16, tag="pt")
                nc.scalar.activation(pta, ta, AF.Exp)
                ptb = ptp.tile([128, 1024], bf16, tag="pt")
                nc.scalar.activation(ptb, tb, AF.Exp)
                for j in range(4):
                    c = 4 * g + j
                    pt_ = pta if j < 2 else ptb
                    c0 = 512 * (j % 2)
                    nc.tensor.matmul(
                        av_acc[32 * j:32 * (j + 1), :],
                        lhsT=vstack[:, D * c:D * (c + 1)],
                        rhs=pt_[:, c0:c0 + 512],
                        start=(g == 0),
                        stop=(g == NG - 1),
                        tile_position=(0, 32 * j),
                        skip_group_check=True,
                    )

            def band_reduce(p, av_acc):
                m0 = p * 512
                t1 = tmp32p.tile([D, 512], f32, tag="t1")
                nc.vector.tensor_copy(t1, av_acc[0:32, :])
                t2 = tmp32p.tile([D, 512], f32, tag="t2")
                nc.vector.tensor_add(t2, t1, av_acc[32:64, :])
                nc.vector.tensor_add(t1, t2, av_acc[64:96, :])
                nc.vector.tensor_add(
                    fusedT[:, m0:m0 + 512], t1, av_acc[96:128, :]
                )

            # ---- LayerNorm + output projection for one 512-row pass ----
            def ln_pass(p):
                mv = lnagg.tile([128, 4, 2], f32, tag="mv")
                cents = []
                for bi in range(4):
                    b = 4 * p + bi
                    fps = pp_ps.tile([128, 512], f32, tag="pp")
                    nc.tensor.transpose(
                        fps[:, 0:D], fusedT[:, b * 128:(b + 1) * 128],
                        ident[0:D, 0:D],
                    )
                    fsb = fbp.tile([128, D], f32, tag="f")
                    nc.vector.tensor_copy(fsb, fps[:, 0:D])
                    st = statp.tile([128, 6], f32, tag="st")
                    nc.vector.bn_stats(out=st, in_=fsb)
                    nc.vector.bn_aggr(out=mv[:, bi, :], in_=st)
                    cent = centp.tile([128, D], f32, tag="c")
                    nc.vector.tensor_scalar(
                        cent, fsb, mv[:, bi, 0:1], None, op0=OP.subtract
                    )
                    cents.append(cent)
                lnv = lnagg.tile([128, 4], f32, tag="lv")
                nc.scalar.activation(lnv, mv[:, :, 1], AF.Ln, bias=epsc)
                rstd = lnagg.tile([128, 4], f32, tag="rs")
                nc.scalar.activation(rstd, lnv, AF.Exp, scale=-0.5)
                for bi in range(4):
                    b = 4 * p + bi
                    nc.vector.tensor_scalar_mul(
                        cents[bi], cents[bi], rstd[:, bi:bi + 1]
                    )
                    npp = pp_ps.tile([128, 512], f32, tag="pp")
                    nc.tensor.transpose(npp[0:D, 0:128], cents[bi], ident)
                    na = na_tiles[bi % 2]
                    nc.vector.tensor_copy(na[0:D, :], npp[0:D, 0:128])
                    ops = pp_ps.tile([128, 512], f32, tag="pp")
                    nc.tensor.matmul(
                        ops[:, 0:DOUT], lhsT=na, rhs=woa_r, start=True, stop=True
                    )
                    osb = outsbp.tile([128, DOUT], f32, tag="o")
                    nc.vector.tensor_copy(osb, ops[:, 0:DOUT])
                    nc.sync.dma_start(out=out_d[b * 128:(b + 1) * 128, :], in_=osb)

            # ---- main: two sequential m-passes ----
            for p in range(NP):
                av_acc = av_ps.tile([128, 512], f32, tag="av", name=f"av{p}")
                for g in range(NG):
                    if p == 0:
                        prep_group(g)
                    attn_group(p, g, av_acc)
                band_reduce(p, av_acc)
                ln_pass(p)

    nc.compile()
    _CACHE["nc"] = nc
    return nc


def host_inputs(input1, input2, Wq, bq, Wk, bk, Wv, bv, gamma, beta, Wo, bo):
    """Per-core input maps (host-side weight folding + bf16 casts)."""
    import ml_dtypes
    f32 = np.float32
    bf16 = ml_dtypes.bfloat16
    input1 = np.ascontiguousarray(np.asarray(input1, f32))
    x2b = np.ascontiguousarray(np.asarray(input2, f32).astype(bf16))
    woa = np.concatenate(
        [np.asarray(gamma, f32)[:, None] * np.asarray(Wo, f32),
         (np.asarray(beta, f32) @ np.asarray(Wo, f32) + np.asarray(bo, f32))[None, :]],
        axis=0,
    ).astype(f32)
    common = {
        "x2b": x2b,
        "wq": np.ascontiguousarray(np.asarray(Wq, f32)),
        "wkb": np.ascontiguousarray(np.asarray(Wk, f32).astype(bf16)),
        "wvb": np.ascontiguousarray(np.asarray(Wv, f32).astype(bf16)),
        "bq4": np.tile(np.asarray(bq, f32), 4)[:, None].copy(),
        "bk4": np.tile(np.asarray(bk, f32), 4)[:, None].copy(),
        "bvb4": np.broadcast_to(np.tile(np.asarray(bv, f32), 4), (128, 128)).copy(),
        "woa": woa,
        "ident": np.eye(128, dtype=f32),
        "epsc": np.full((128, 1), LN_EPS, f32),
    }
    return [
        dict(common, x1=input1[c * MSH:(c + 1) * MSH]) for c in range(NCORES)
    ]


def kernel(input1, input2, Wq, bq, Wk, bk, Wv, bv, gamma, beta, Wo, bo):
    _import_concourse()
    from concourse.bass_utils import run_bass_kernel_spmd

    nc = build()
    in_maps = host_inputs(
        input1, input2, Wq, bq, Wk, bk, Wv, bv, gamma, beta, Wo, bo
    )
    res = run_bass_kernel_spmd(nc, in_maps, list(range(NCORES)))
    return np.concatenate(
        [np.asarray(res.results[c]["out"]) for c in range(NCORES)], axis=0
    ).astype(np.float32)


# revision 5
# speedup vs baseline: 1.3055x; 1.1939x over previous
"""Trainium2 Bass kernel for fused attention + LayerNorm + projection.

Computation (per reference):
    q = input1 @ Wq + bq                       [8192, 32]
    k = input2 @ Wk + bk                       [8192, 32]
    v = input2 @ Wv + bv                       [8192, 32]
    P = softmax(q @ k.T, axis=-1)              [8192, 8192]
    fused = P @ v                              [8192, 32]
    out = LayerNorm(fused) * gamma + beta @ Wo + bo   [8192, 128]

Sharding: data-parallel over rows of input1 (1024 rows per core, 8 cores);
input2 and weights replicated.

Algebraic simplifications (validated vs reference, rel err ~6e-3):
  - softmax normalization (and max-subtraction) skipped: LayerNorm is
    invariant to a positive per-row scale, so exp(s) @ v is normalized for
    free by LN.
  - gamma folded into Wo (diag(gamma) @ Wo), beta/bo folded into an extra
    contraction row via an augmented ones-row, on the host.

Dataflow per core (ACT-exp is the roofline: 8.4M exps at 1 elem/lane/cyc
@1.2GHz ~= 64us; everything else is arranged to overlap under it):
  - x2 is cast to bf16 on host and DMA'd HBM->SBUF through the XBAR
    transpose unit: x2T [128d, 8192n] lands directly with the contraction
    dim on partitions - zero PE transposes for the k/v projections.
  - prep is batched 4 groups (16 chunks) per PSUM tile: 4x 4-way
    column-tiled k quads into bank0 (kT banded into kstack f32r, chunk c
    at partitions 32*(c%4)) and 16 v matmuls into bank1 (v natural
    [n,32]), then ONE bias-add each for kstack/vstack (DVE).
  - q: x1 stays f32 for precision; PE-transposed, then 4-way col-tiled
    matmul replicates qT into all 4 partition bands of qt_rep f32r.
  - scores: per (pass, group) ONE 4-way row-tiled f32r matmul quad
    (tile_position=(32j,0), concurrent on the PE sub-arrays) writes 4
    PSUM banks = scoresT for 4 chunks x 512 m.
  - exp on ACT straight out of PSUM, 2x [128,1024] bf16 -> pt.
  - AV: 4-way col-tiled bf16 quad accumulates fusedT into one PSUM bank
    across all 16 groups (start on g==0, stop on g==15).
  - m is processed in 2 sequential passes of 512 rows; pass 0's LayerNorm
    + output projection overlaps pass 1's attention stream.
  - PSUM: 3x [128,1024] score/prep tiles (6 banks) + 1 AV bank + 1
    prep/LN bank = 8.
  - LayerNorm (batched, 4 blocks per pass): bn_stats/bn_aggr;
    rstd = exp(-0.5*ln(var+eps)) keeps the ACT table in the ln/exp
    family (no table switch); projection through an augmented [33,512]
    lhsT with a persistent ones row; one rearranged output DMA per pass.
"""

import os
import sys

import numpy as np

N1 = 8192
N2 = 8192
DIN = 128
D = 32
DOUT = 128
NCORES = 8
MSH = N1 // NCORES          # rows per core
NCH = N2 // 128             # 64 in2 chunks
NG = NCH // 4               # 16 groups of 4 chunks
NB = NG // 4                # 4 prep batches of 4 groups
NP = MSH // 512             # 2 m-passes of 512 rows
LN_EPS = 1e-5

_CACHE = {}


def _import_concourse():
    try:
        import concourse.bass  # noqa: F401
    except ImportError:
        for p in ("/opt/trn_rl_repo", os.path.expanduser("~/.axon_site/_ro/trn_rl_repo")):
            if os.path.isdir(p) and p not in sys.path:
                sys.path.insert(0, p)


def build():
    """Build (and cache) the compiled single-core SPMD Bass program."""
    if "nc" in _CACHE:
        return _CACHE["nc"]
    _import_concourse()
    import concourse.bacc as bacc
    import concourse.tile as tile
    from concourse import mybir

    f32 = mybir.dt.float32
    f32r = mybir.dt.float32r
    bf16 = mybir.dt.bfloat16
    AF = mybir.ActivationFunctionType
    OP = mybir.AluOpType

    nc = bacc.Bacc(None, target_bir_lowering=False, debug=False)

    x1 = nc.dram_tensor("x1", [MSH, DIN], f32, kind="ExternalInput")
    x2b = nc.dram_tensor("x2b", [N2, DIN], bf16, kind="ExternalInput")
    wq_d = nc.dram_tensor("wq", [DIN, D], f32, kind="ExternalInput")
    wkb_d = nc.dram_tensor("wkb", [DIN, D], bf16, kind="ExternalInput")
    wvb_d = nc.dram_tensor("wvb", [DIN, D], bf16, kind="ExternalInput")
    bq4_d = nc.dram_tensor("bq4", [128, 1], f32, kind="ExternalInput")
    bk4_d = nc.dram_tensor("bk4", [128, 1], f32, kind="ExternalInput")
    bv16_d = nc.dram_tensor("bv16", [128, 512], f32, kind="ExternalInput")
    woa_d = nc.dram_tensor("woa", [D + 1, DOUT], f32, kind="ExternalInput")
    id_d = nc.dram_tensor("ident", [128, 128], f32, kind="ExternalInput")
    eps_d = nc.dram_tensor("epsc", [128, 1], f32, kind="ExternalInput")
    out_d = nc.dram_tensor("out", [MSH, DOUT], f32, kind="ExternalOutput")

    from contextlib import ExitStack

    with tile.TileContext(nc) as tc, ExitStack() as outer:
        consts = outer.enter_context(tc.tile_pool(name="consts", bufs=1))
        persist = outer.enter_context(tc.tile_pool(name="persist", bufs=1))

        ident = consts.tile([128, 128], f32)
        nc.sync.dma_start(out=ident, in_=id_d[:])
        wq = consts.tile([DIN, D], f32)
        nc.sync.dma_start(out=wq, in_=wq_d[:])
        wk = consts.tile([DIN, D], bf16)
        nc.sync.dma_start(out=wk, in_=wkb_d[:])
        wv = consts.tile([DIN, D], bf16)
        nc.sync.dma_start(out=wv, in_=wvb_d[:])
        bq4 = consts.tile([128, 1], f32)
        nc.sync.dma_start(out=bq4, in_=bq4_d[:])
        bk4 = consts.tile([128, 1], f32)
        nc.sync.dma_start(out=bk4, in_=bk4_d[:])
        bv16 = consts.tile([128, 512], f32)
        nc.sync.dma_start(out=bv16, in_=bv16_d[:])
        woa = consts.tile([D + 1, DOUT], f32)
        nc.sync.dma_start(out=woa, in_=woa_d[:])
        epsc = consts.tile([128, 1], f32)
        nc.sync.dma_start(out=epsc, in_=eps_d[:])
        woa_r = consts.tile([D + 1, DOUT], f32r)
        nc.vector.tensor_copy(woa_r, woa)

        # Pull the exp table load (~2.7us) into the initial DMA window.
        warm = consts.tile([1, 8], f32)
        nc.scalar.activation(warm, ident[0:1, 0:8], AF.Exp)

        x2t = persist.tile([128, N2], bf16)             # x2 transposed (d on partitions)
        kstack = persist.tile([128, NG * 128], f32r)    # kT chunk c: [32*(c%4):+32, 128*(c//4):+128]
        vstack = persist.tile([128, NCH * D], bf16)     # v chunk c: [:, 32*c:+32]
        qt_rep = persist.tile([128, MSH], f32r)         # qT replicated in 4 bands
        x1t_all = persist.tile([128, MSH], f32)         # input1 shard transposed
        fusedT = persist.tile([D, MSH], f32)            # v.T @ P.T after band reduce
        na_all = persist.tile([D + 1, 512], f32r)       # augmented LN lhsT; row D = ones

        with (
            tc.tile_pool(name="sc_ps", bufs=3, space="PSUM") as sc_ps,
            tc.tile_pool(name="av_ps", bufs=1, space="PSUM") as av_ps,
            tc.tile_pool(name="pp_ps", bufs=1, space="PSUM") as pp_ps,
            tc.tile_pool(name="x1load", bufs=1) as x1load,
            tc.tile_pool(name="pt", bufs=4) as ptp,
            tc.tile_pool(name="tmp32", bufs=2) as tmp32p,
            tc.tile_pool(name="fb", bufs=2) as fbp,
            tc.tile_pool(name="cent", bufs=2) as centp,
            tc.tile_pool(name="stat", bufs=2) as statp,
            tc.tile_pool(name="lnagg", bufs=2) as lnagg,
            tc.tile_pool(name="outsb", bufs=2) as outsbp,
        ):
            nc.vector.tensor_scalar(
                na_all[D:D + 1, :], bv16[0:1, :], 0.0, 1.0,
                op0=OP.mult, op1=OP.add,
            )

            # ---- input DMAs: x1 first (q prep gates the stream start) ----
            x1_sb = x1load.tile([128, MSH // 128, 128], f32)
            nc.sync.dma_start(
                out=x1_sb, in_=x1[:].rearrange("(t p) d -> p t d", p=128)
            )
            for g in range(NG):
                nc.sync.dma_start(
                    out=x2t[:, g * 512:(g + 1) * 512],
                    in_=x2b[g * 512:(g + 1) * 512, :],
                    transpose=True,
                )

            # ---- q prep (through sc tiles; pp stays free for LN) ----
            tsp = sc_ps.tile([128, 1024], f32, tag="sc")
            for t in range(MSH // 128):
                nc.tensor.transpose(
                    tsp[:, (t % 8) * 128:(t % 8) * 128 + 128], x1_sb[:, t, :], ident
                )
                nc.vector.tensor_copy(
                    x1t_all[:, t * 128:(t + 1) * 128],
                    tsp[:, (t % 8) * 128:(t % 8) * 128 + 128],
                )
            qsp = sc_ps.tile([128, 1024], f32, tag="sc")
            for t2 in range(MSH // 512):
                for j in range(4):
                    nc.tensor.matmul(
                        qsp[32 * j:32 * (j + 1), t2 * 512:(t2 + 1) * 512],
                        lhsT=wq,
                        rhs=x1t_all[:, t2 * 512:(t2 + 1) * 512],
                        start=True,
                        stop=True,
                        tile_position=(0, 32 * j),
                    )
                nc.vector.tensor_scalar_add(
                    qt_rep[:, t2 * 512:(t2 + 1) * 512],
                    qsp[:, t2 * 512:(t2 + 1) * 512],
                    bq4,
                )

            # ---- k/v prep, batched: 4 groups (16 chunks) per PSUM tile ----
            def prep_batch(B):
                pp = sc_ps.tile([128, 1024], f32, tag="sc")
                for gi in range(4):
                    g = 4 * B + gi
                    for j in range(4):
                        nc.tensor.matmul(
                            pp[32 * j:32 * (j + 1), gi * 128:(gi + 1) * 128],
                            lhsT=wk,
                            rhs=x2t[:, (4 * g + j) * 128:(4 * g + j + 1) * 128],
                            start=True,
                            stop=True,
                            tile_position=(0, 32 * j),
                        )
                for ci in range(16):
                    c = 16 * B + ci
                    nc.tensor.matmul(
                        pp[:, 512 + 32 * ci:512 + 32 * (ci + 1)],
                        lhsT=x2t[:, c * 128:(c + 1) * 128],
                        rhs=wv,
                        start=True,
                        stop=True,
                    )
                nc.vector.tensor_scalar_add(
                    kstack[:, B * 512:(B + 1) * 512], pp[:, 0:512], bk4
                )
                nc.vector.tensor_add(
                    vstack[:, B * 512:(B + 1) * 512], pp[:, 512:1024], bv16
                )

            # ---- one attention group: 4-way score quad, 2 exps, 4-way AV quad ----
            def attn_group(p, g, av_acc):
                m0 = p * 512
                ta = sc_ps.tile([128, 1024], f32, tag="sc")
                tb = sc_ps.tile([128, 1024], f32, tag="sc")
                for j in range(4):
                    t_ = ta if j < 2 else tb
                    c0 = 512 * (j % 2)
                    nc.tensor.matmul(
                        t_[:, c0:c0 + 512],
                        lhsT=kstack[32 * j:32 * (j + 1), g * 128:(g + 1) * 128],
                        rhs=qt_rep[32 * j:32 * (j + 1), m0:m0 + 512],
                        start=True,
                        stop=True,
                        tile_position=(32 * j, 0),
                    )
                pta = ptp.tile([128, 1024], bf16, tag="pt")
                nc.scalar.activation(pta, ta, AF.Exp)
                ptb = ptp.tile([128, 1024], bf16, tag="pt")
                nc.scalar.activation(ptb, tb, AF.Exp)
                for j in range(4):
                    c = 4 * g + j
                    pt_ = pta if j < 2 else ptb
                    c0 = 512 * (j % 2)
                    nc.tensor.matmul(
                        av_acc[32 * j:32 * (j + 1), :],
                        lhsT=vstack[:, D * c:D * (c + 1)],
                        rhs=pt_[:, c0:c0 + 512],
                        start=(g == 0),
                        stop=(g == NG - 1),
                        tile_position=(0, 32 * j),
                        skip_group_check=True,
                    )

            def band_reduce(p, av_acc):
                m0 = p * 512
                t1 = tmp32p.tile([D, 512], f32, tag="t1")
                nc.vector.tensor_copy(t1, av_acc[0:32, :])
                t2 = tmp32p.tile([D, 512], f32, tag="t2")
                nc.vector.tensor_add(t2, t1, av_acc[32:64, :])
                nc.vector.tensor_add(t1, t2, av_acc[64:96, :])
                nc.vector.tensor_add(
                    fusedT[:, m0:m0 + 512], t1, av_acc[96:128, :]
                )

            # ---- LayerNorm + output projection for one 512-row pass ----
            def ln_pass(p):
                m0 = p * 512
                # T1: all 4 block transposes into one bank
                fp1 = pp_ps.tile([128, 512], f32, tag="pp")
                for bi in range(4):
                    nc.tensor.transpose(
                        fp1[:, 32 * bi:32 * (bi + 1)],
                        fusedT[:, m0 + bi * 128:m0 + (bi + 1) * 128],
                        ident[0:D, 0:D],
                    )
                fsb = fbp.tile([128, 128], f32, tag="f")
                nc.vector.tensor_copy(fsb, fp1[:, 0:128])
                mv = lnagg.tile([128, 4, 2], f32, tag="mv")
                for bi in range(4):
                    st = statp.tile([128, 6], f32, tag="st")
                    nc.vector.bn_stats(out=st, in_=fsb[:, 32 * bi:32 * (bi + 1)])
                    nc.vector.bn_aggr(out=mv[:, bi, :], in_=st)
                lnv = lnagg.tile([128, 4], f32, tag="lv")
                nc.scalar.activation(lnv, mv[:, :, 1], AF.Ln, bias=epsc)
                rstd = lnagg.tile([128, 4], f32, tag="rs")
                nc.scalar.activation(rstd, lnv, AF.Exp, scale=-0.5)
                cent = centp.tile([128, 128], f32, tag="c")
                for bi in range(4):
                    nc.vector.tensor_scalar(
                        cent[:, 32 * bi:32 * (bi + 1)],
                        fsb[:, 32 * bi:32 * (bi + 1)],
                        mv[:, bi, 0:1], rstd[:, bi:bi + 1],
                        op0=OP.subtract, op1=OP.mult,
                    )
                # T2: transpose normalized blocks back to [32, 128] each
                fp2 = pp_ps.tile([128, 512], f32, tag="pp")
                for bi in range(4):
                    nc.tensor.transpose(
                        fp2[0:D, 128 * bi:128 * (bi + 1)],
                        cent[:, 32 * bi:32 * (bi + 1)],
                        ident,
                    )
                nc.vector.tensor_copy(na_all[0:D, :], fp2[0:D, 0:512])
                # T3: augmented projection matmuls
                fp3 = pp_ps.tile([128, 512], f32, tag="pp")
                for bi in range(4):
                    nc.tensor.matmul(
                        fp3[:, 128 * bi:128 * (bi + 1)],
                        lhsT=na_all[:, 128 * bi:128 * (bi + 1)],
                        rhs=woa_r,
                        start=True,
                        stop=True,
                    )
                osb = outsbp.tile([128, 4, DOUT], f32, tag="o")
                nc.vector.tensor_copy(osb, fp3[:].rearrange("p (t d) -> p t d", t=4))
                nc.sync.dma_start(
                    out=out_d[m0:m0 + 512, :].rearrange("(t p) d -> p t d", p=128),
                    in_=osb,
                )

            # ---- main: two sequential m-passes ----
            for p in range(NP):
                av_acc = av_ps.tile([128, 512], f32, tag="av", name=f"av{p}")
                for g in range(NG):
                    if p == 0 and g % 4 == 0:
                        prep_batch(g // 4)
                    attn_group(p, g, av_acc)
                band_reduce(p, av_acc)
                ln_pass(p)

    nc.compile()
    _CACHE["nc"] = nc
    return nc


def host_inputs(input1, input2, Wq, bq, Wk, bk, Wv, bv, gamma, beta, Wo, bo):
    """Per-core input maps (host-side weight folding + bf16 casts)."""
    import ml_dtypes
    f32 = np.float32
    bf16 = ml_dtypes.bfloat16
    input1 = np.ascontiguousarray(np.asarray(input1, f32))
    x2b = np.ascontiguousarray(np.asarray(input2, f32).astype(bf16))
    woa = np.concatenate(
        [np.asarray(gamma, f32)[:, None] * np.asarray(Wo, f32),
         (np.asarray(beta, f32) @ np.asarray(Wo, f32) + np.asarray(bo, f32))[None, :]],
        axis=0,
    ).astype(f32)
    common = {
        "x2b": x2b,
        "wq": np.ascontiguousarray(np.asarray(Wq, f32)),
        "wkb": np.ascontiguousarray(np.asarray(Wk, f32).astype(bf16)),
        "wvb": np.ascontiguousarray(np.asarray(Wv, f32).astype(bf16)),
        "bq4": np.tile(np.asarray(bq, f32), 4)[:, None].copy(),
        "bk4": np.tile(np.asarray(bk, f32), 4)[:, None].copy(),
        "bv16": np.broadcast_to(np.tile(np.asarray(bv, f32), 16), (128, 512)).copy(),
        "woa": woa,
        "ident": np.eye(128, dtype=f32),
        "epsc": np.full((128, 1), LN_EPS, f32),
    }
    return [
        dict(common, x1=input1[c * MSH:(c + 1) * MSH]) for c in range(NCORES)
    ]


def kernel(input1, input2, Wq, bq, Wk, bk, Wv, bv, gamma, beta, Wo, bo):
    _import_concourse()
    from concourse.bass_utils import run_bass_kernel_spmd

    nc = build()
    in_maps = host_inputs(
        input1, input2, Wq, bq, Wk, bk, Wv, bv, gamma, beta, Wo, bo
    )
    res = run_bass_kernel_spmd(nc, in_maps, list(range(NCORES)))
    return np.concatenate(
        [np.asarray(res.results[c]["out"]) for c in range(NCORES)], axis=0
    ).astype(np.float32)


# revision 8
# speedup vs baseline: 1.3439x; 1.0294x over previous
"""Trainium2 Bass kernel for fused attention + LayerNorm + projection.

Computation (per reference):
    q = input1 @ Wq + bq                       [8192, 32]
    k = input2 @ Wk + bk                       [8192, 32]
    v = input2 @ Wv + bv                       [8192, 32]
    P = softmax(q @ k.T, axis=-1)              [8192, 8192]
    fused = P @ v                              [8192, 32]
    out = LayerNorm(fused) * gamma + beta @ Wo + bo   [8192, 128]

Sharding: data-parallel over rows of input1 (1024 rows per core, 8 cores);
input2 and weights replicated.

Algebraic simplifications (validated vs reference, rel err ~6e-3):
  - softmax normalization (and max-subtraction) skipped: LayerNorm is
    invariant to a positive per-row scale, so exp(s) @ v is normalized for
    free by LN.
  - gamma folded into Wo (diag(gamma) @ Wo), beta/bo folded into an extra
    contraction row via an augmented ones-row, on the host.

Dataflow per core (ACT-exp is the roofline: 8.4M exps at 1 elem/lane/cyc
@1.2GHz ~= 64us; everything else is arranged to overlap under it):
  - x2 is cast to bf16 on host and DMA'd HBM->SBUF through the XBAR
    transpose unit: x2T [128d, 8192n] lands directly with the contraction
    dim on partitions - zero PE transposes for the k/v projections.
  - prep is batched 4 groups (16 chunks) per dedicated 2-bank PSUM tile:
    4x 4-way column-tiled k quads into bank0 (kT banded into kstack f32r,
    chunk c at partitions 32*(c%4)) and 16 v matmuls into bank1 (v
    natural [n,32] -> vstack bf16), then ONE bias-add each. All 4 batches
    are emitted up front so prep races ahead of the attention stream
    during its PE-idle ramp.
  - q: x1 stays f32 for precision; PE-transposed (bank-alternated), then
    a 4-way col-tiled f32r matmul replicates qT into all 4 partition
    bands of qt_rep (fp32 matmuls would split into HI/LO pairs and run
    4x slower - everything on the PE is f32r or bf16).
  - scores: per (pass, group) ONE 4-way row-tiled f32r matmul quad
    (tile_position=(32j,0), concurrent on the PE sub-arrays) writes 4
    PSUM banks = scoresT for 4 chunks x 512 m.
  - exp on ACT straight out of PSUM, 2x [128,1024] bf16 -> pt. Emission
    is software-pipelined (scores g | exp g | AV g-1) so the PE finishes
    each score quad inside the previous exp's shadow and ACT never gaps.
  - AV: 4-way col-tiled bf16 quad accumulates fusedT into one PSUM bank
    across all 16 groups (start on g==0, stop on g==15).
  - m is processed in 2 sequential passes of 512 rows; pass 0's LayerNorm
    + output projection overlaps pass 1's attention stream.
  - PSUM: 2x [128,1024] score tiles (4 banks) + prep tile (2 banks) +
    1 AV bank + 1 LN bank = 8.
  - LayerNorm (batched, 4 blocks per pass): bn_stats/bn_aggr; rstd is
    computed on the DVE with the bit-trick rsqrt + 2 Newton steps so the
    ACT engine NEVER leaves the exp table set (a Ln/Sqrt activation would
    cost two ~2.7us ACT_TABLE_LOAD+DRAIN stalls per pass); projection
    through an augmented [33,512] lhsT with a persistent ones row; one
    rearranged output DMA per pass.
"""

import os
import sys

import numpy as np

N1 = 8192
N2 = 8192
DIN = 128
D = 32
DOUT = 128
NCORES = 8
MSH = N1 // NCORES          # rows per core
NCH = N2 // 128             # 64 in2 chunks
NG = NCH // 4               # 16 groups of 4 chunks
NB = NG // 4                # 4 prep batches of 4 groups
NP = MSH // 512             # 2 m-passes of 512 rows
LN_EPS = 1e-5

_CACHE = {}


def _import_concourse():
    try:
        import concourse.bass  # noqa: F401
    except ImportError:
        for p in ("/opt/trn_rl_repo", os.path.expanduser("~/.axon_site/_ro/trn_rl_repo")):
            if os.path.isdir(p) and p not in sys.path:
                sys.path.insert(0, p)


# Packed f32 consts layout (columns of the [128, CW] "cst" tensor).
C_ID = 0          # ident [128,128]
C_BV = 128        # bv16  [128,512]
C_BQ = 640        # bq4   [128,1]
C_BK = 641        # bk4   [128,1]
C_EPS = 642       # epsc  [128,1]
C_WOA = 643       # woa   [33,128] (rows 0:33)
C_WQ = 771        # wq4   [128,128] (Wq tiled 4x in cols)
CW = 899


def build():
    """Build (and cache) the compiled single-core SPMD Bass program."""
    if "nc" in _CACHE:
        return _CACHE["nc"]
    _import_concourse()
    import concourse.bacc as bacc
    import concourse.tile as tile
    from concourse import mybir

    f32 = mybir.dt.float32
    f32r = mybir.dt.float32r
    i32 = mybir.dt.int32
    bf16 = mybir.dt.bfloat16
    AF = mybir.ActivationFunctionType
    OP = mybir.AluOpType

    nc = bacc.Bacc(None, target_bir_lowering=False, debug=False)

    x1 = nc.dram_tensor("x1", [MSH, DIN], f32, kind="ExternalInput")
    x2b = nc.dram_tensor("x2b", [N2, DIN], bf16, kind="ExternalInput")
    cst_d = nc.dram_tensor("cst", [128, CW], f32, kind="ExternalInput")
    wkv_d = nc.dram_tensor("wkv", [DIN, 2 * D], bf16, kind="ExternalInput")
    out_d = nc.dram_tensor("out", [MSH, DOUT], f32, kind="ExternalOutput")

    from contextlib import ExitStack

    with tile.TileContext(nc) as tc, ExitStack() as outer:
        consts = outer.enter_context(tc.tile_pool(name="consts", bufs=1))
        persist = outer.enter_context(tc.tile_pool(name="persist", bufs=1))

        cst = consts.tile([128, CW], f32)
        nc.sync.dma_start(out=cst, in_=cst_d[:])
        wkv = consts.tile([DIN, 2 * D], bf16)
        nc.sync.dma_start(out=wkv, in_=wkv_d[:])
        ident = cst[:, C_ID:C_ID + 128]
        bv16 = cst[:, C_BV:C_BV + 512]
        bq4 = cst[:, C_BQ:C_BQ + 1]
        bk4 = cst[:, C_BK:C_BK + 1]
        epsc = cst[:, C_EPS:C_EPS + 1]
        wk = wkv[:, 0:D]
        wv = wkv[:, D:2 * D]

        woa_r = consts.tile([D + 1, DOUT], f32r)
        nc.vector.tensor_copy(woa_r, cst[0:D + 1, C_WOA:C_WOA + 128])
        wq4_r = consts.tile([DIN, 128], f32r)
        nc.vector.tensor_copy(wq4_r, cst[:, C_WQ:C_WQ + 128])
        magic = consts.tile([128, NP * 4], i32)
        nc.vector.memset(magic, 0x5F3759DF)

        # Pull the exp table load (~2.7us) into the initial DMA window.
        warm = consts.tile([1, 8], f32)
        nc.scalar.activation(warm, cst[0:1, 0:8], AF.Exp)

        x2t = persist.tile([128, N2], bf16)             # x2 transposed (d on partitions)
        kstack = persist.tile([128, NG * 128], f32r)    # kT chunk c: [32*(c%4):+32, 128*(c//4):+128]
        vstack = persist.tile([128, NCH * D], bf16)     # v chunk c: [:, 32*c:+32]
        qt_rep = persist.tile([128, MSH], f32r)         # qT replicated in 4 bands
        x1t_all = persist.tile([128, MSH], f32r)        # input1 shard transposed
        fusedT = persist.tile([D, MSH], f32)            # v.T @ P.T after band reduce
        na_all = persist.tile([D + 1, 512], f32r)       # augmented LN lhsT; row D = ones

        with (
            tc.tile_pool(name="sc_ps", bufs=2, space="PSUM") as sc_ps,
            tc.tile_pool(name="kv_ps", bufs=1, space="PSUM") as kv_ps,
            tc.tile_pool(name="av_ps", bufs=1, space="PSUM") as av_ps,
            tc.tile_pool(name="pp_ps", bufs=1, space="PSUM") as pp_ps,
            tc.tile_pool(name="x1load", bufs=1) as x1load,
            tc.tile_pool(name="pt", bufs=4) as ptp,
            tc.tile_pool(name="tmp32", bufs=2) as tmp32p,
            tc.tile_pool(name="fb", bufs=2) as fbp,
            tc.tile_pool(name="cent", bufs=2) as centp,
            tc.tile_pool(name="stat", bufs=2) as statp,
            tc.tile_pool(name="lnagg", bufs=8) as lnagg,
            tc.tile_pool(name="outsb", bufs=2) as outsbp,
        ):
            nc.vector.tensor_scalar(
                na_all[D:D + 1, :], bv16[0:1, :], 0.0, 1.0,
                op0=OP.mult, op1=OP.add,
            )

            # ---- input DMAs: x1 first (q prep gates the stream start) ----
            x1_sb = x1load.tile([128, MSH // 128, 128], f32)
            nc.sync.dma_start(
                out=x1_sb, in_=x1[:].rearrange("(t p) d -> p t d", p=128)
            )
            for g in range(NG):
                nc.sync.dma_start(
                    out=x2t[:, g * 512:(g + 1) * 512],
                    in_=x2b[g * 512:(g + 1) * 512, :],
                    transpose=True,
                )

            # ---- q prep (through sc tiles; transposes alternate banks) ----
            tsp = sc_ps.tile([128, 1024], f32, tag="sc")
            for t in range(MSH // 128):
                col = (t % 2) * 512 + (t // 2) * 128
                nc.tensor.transpose(tsp[:, col:col + 128], x1_sb[:, t, :], ident)
            for h in range(2):
                nc.vector.tensor_copy(
                    x1t_all[:].rearrange("p (t d) -> p t d", d=128)[:, h::2, :],
                    tsp[:, h * 512:(h + 1) * 512].rearrange(
                        "p (t d) -> p t d", d=128
                    ),
                )
            qsp = sc_ps.tile([128, 1024], f32, tag="sc")
            for t2 in range(MSH // 512):
                nc.tensor.matmul(
                    qsp[:, t2 * 512:(t2 + 1) * 512],
                    lhsT=wq4_r,
                    rhs=x1t_all[:, t2 * 512:(t2 + 1) * 512],
                    start=True,
                    stop=True,
                )
                nc.vector.tensor_scalar_add(
                    qt_rep[:, t2 * 512:(t2 + 1) * 512],
                    qsp[:, t2 * 512:(t2 + 1) * 512],
                    bq4,
                )

            # ---- k/v prep: 4 batches of 4 groups, all emitted up front ----
            def prep_batch(B):
                pp = kv_ps.tile([128, 1024], f32, tag="kv")
                for gi in range(4):
                    g = 4 * B + gi
                    for j in range(4):
                        nc.tensor.matmul(
                            pp[32 * j:32 * (j + 1), gi * 128:(gi + 1) * 128],
                            lhsT=wk,
                            rhs=x2t[:, (4 * g + j) * 128:(4 * g + j + 1) * 128],
                            start=True,
                            stop=True,
                            tile_position=(0, 32 * j),
                        )
                nc.vector.tensor_scalar_add(
                    kstack[:, B * 512:(B + 1) * 512], pp[:, 0:512], bk4
                )
                for ci in range(16):
                    c = 16 * B + ci
                    nc.tensor.matmul(
                        pp[:, 512 + 32 * ci:512 + 32 * (ci + 1)],
                        lhsT=x2t[:, c * 128:(c + 1) * 128],
                        rhs=wv,
                        start=True,
                        stop=True,
                    )
                nc.vector.tensor_add(
                    vstack[:, B * 512:(B + 1) * 512], pp[:, 512:1024], bv16
                )

            for B in range(NB):
                prep_batch(B)

            # ---- attention stream pieces ----
            def score_exp(p, g):
                m0 = p * 512
                ta = sc_ps.tile([128, 1024], f32, tag="sc")
                tb = sc_ps.tile([128, 1024], f32, tag="sc")
                for j in range(4):
                    t_ = ta if j < 2 else tb
                    c0 = 512 * (j % 2)
                    nc.tensor.matmul(
                        t_[:, c0:c0 + 512],
                        lhsT=kstack[32 * j:32 * (j + 1), g * 128:(g + 1) * 128],
                        rhs=qt_rep[32 * j:32 * (j + 1), m0:m0 + 512],
                        start=True,
                        stop=True,
                        tile_position=(32 * j, 0),
                    )
                pta = ptp.tile([128, 1024], bf16, tag="pt")
                nc.scalar.activation(pta, ta, AF.Exp)
                ptb = ptp.tile([128, 1024], bf16, tag="pt")
                nc.scalar.activation(ptb, tb, AF.Exp)
                return pta, ptb

            def av_quad(g, pts, av_acc):
                pta, ptb = pts
                for j in range(4):
                    c = 4 * g + j
                    pt_ = pta if j < 2 else ptb
                    c0 = 512 * (j % 2)
                    nc.tensor.matmul(
                        av_acc[32 * j:32 * (j + 1), :],
                        lhsT=vstack[:, D * c:D * (c + 1)],
                        rhs=pt_[:, c0:c0 + 512],
                        start=(g == 0),
                        stop=(g == NG - 1),
                        tile_position=(0, 32 * j),
                        skip_group_check=True,
                    )

            def band_reduce(p, av_acc):
                m0 = p * 512
                t1 = tmp32p.tile([D, 512], f32, tag="t1")
                nc.vector.tensor_copy(t1, av_acc[0:32, :])
                t2 = tmp32p.tile([D, 512], f32, tag="t2")
                nc.vector.tensor_add(t2, t1, av_acc[32:64, :])
                nc.vector.tensor_add(t1, t2, av_acc[64:96, :])
                nc.vector.tensor_add(
                    fusedT[:, m0:m0 + 512], t1, av_acc[96:128, :]
                )

            def rsqrt_dve(rstd, ve, p):
                """rstd = ve**-0.5 on the DVE (bit-trick seed + 2 Newton)."""
                sh = lnagg.tile([128, 4], i32, tag="sh")
                nc.vector.tensor_scalar(
                    sh, ve.bitcast(i32), 1, None, op0=OP.logical_shift_right
                )
                s_i = lnagg.tile([128, 4], i32, tag="si")
                nc.vector.tensor_tensor(
                    s_i, magic[:, 4 * p:4 * (p + 1)], sh, op=OP.subtract
                )
                cur = s_i.bitcast(f32)
                for it in range(2):
                    a = lnagg.tile([128, 4], f32, tag=f"a{it}")
                    nc.vector.tensor_tensor(a, cur, cur, op=OP.mult)
                    nc.vector.tensor_tensor(a, a, ve, op=OP.mult)
                    nc.vector.tensor_scalar(
                        a, a, -0.5, 1.5, op0=OP.mult, op1=OP.add
                    )
                    nxt = rstd if it == 1 else lnagg.tile([128, 4], f32, tag="s1")
                    nc.vector.tensor_tensor(nxt, cur, a, op=OP.mult)
                    cur = nxt

            # ---- LayerNorm + output projection for one 512-row pass ----
            def ln_pass(p):
                m0 = p * 512
                fp1 = pp_ps.tile([128, 512], f32, tag="pp")
                for bi in range(4):
                    nc.tensor.transpose(
                        fp1[:, 32 * bi:32 * (bi + 1)],
                        fusedT[:, m0 + bi * 128:m0 + (bi + 1) * 128],
                        ident[0:D, 0:D],
                    )
                fsb = fbp.tile([128, 128], f32, tag="f")
                nc.vector.tensor_copy(fsb, fp1[:, 0:128])
                mv = lnagg.tile([128, 4, 2], f32, tag="mv")
                for bi in range(4):
                    st = statp.tile([128, 6], f32, tag="st")
                    nc.vector.bn_stats(out=st, in_=fsb[:, 32 * bi:32 * (bi + 1)])
                    nc.vector.bn_aggr(out=mv[:, bi, :], in_=st)
                ve = lnagg.tile([128, 4], f32, tag="ve")
                nc.vector.tensor_scalar_add(ve, mv[:, :, 1], epsc)
                rstd = lnagg.tile([128, 4], f32, tag="rs")
                rsqrt_dve(rstd, ve, p)
                cent = centp.tile([128, 128], f32, tag="c")
                for bi in range(4):
                    nc.vector.tensor_scalar(
                        cent[:, 32 * bi:32 * (bi + 1)],
                        fsb[:, 32 * bi:32 * (bi + 1)],
                        mv[:, bi, 0:1], rstd[:, bi:bi + 1],
                        op0=OP.subtract, op1=OP.mult,
                    )
                fp2 = pp_ps.tile([128, 512], f32, tag="pp")
                for bi in range(4):
                    nc.tensor.transpose(
                        fp2[0:D, 128 * bi:128 * (bi + 1)],
                        cent[:, 32 * bi:32 * (bi + 1)],
                        ident,
                    )
                nc.vector.tensor_copy(na_all[0:D, :], fp2[0:D, 0:512])
                fp3 = pp_ps.tile([128, 512], f32, tag="pp")
                for bi in range(4):
                    nc.tensor.matmul(
                        fp3[:, 128 * bi:128 * (bi + 1)],
                        lhsT=na_all[:, 128 * bi:128 * (bi + 1)],
                        rhs=woa_r,
                        start=True,
                        stop=True,
                    )
                osb = outsbp.tile([128, 4, DOUT], f32, tag="o")
                nc.vector.tensor_copy(osb, fp3[:].rearrange("p (t d) -> p t d", t=4))
                nc.sync.dma_start(
                    out=out_d[m0:m0 + 512, :].rearrange("(t p) d -> p t d", p=128),
                    in_=osb,
                )

            # ---- main: two sequential m-passes, software-pipelined AV ----
            for p in range(NP):
                av_acc = av_ps.tile([128, 512], f32, tag="av", name=f"av{p}")
                pts = None
                for g in range(NG):
                    new_pts = score_exp(p, g)
                    if pts is not None:
                        av_quad(g - 1, pts, av_acc)
                    pts = new_pts
                av_quad(NG - 1, pts, av_acc)
                band_reduce(p, av_acc)
                ln_pass(p)

    nc.compile()
    _CACHE["nc"] = nc
    return nc


def host_inputs(input1, input2, Wq, bq, Wk, bk, Wv, bv, gamma, beta, Wo, bo):
    """Per-core input maps (host-side weight folding + bf16 casts)."""
    import ml_dtypes
    f32 = np.float32
    bf16 = ml_dtypes.bfloat16
    input1 = np.ascontiguousarray(np.asarray(input1, f32))
    x2b = np.ascontiguousarray(np.asarray(input2, f32).astype(bf16))
    woa = np.concatenate(
        [np.asarray(gamma, f32)[:, None] * np.asarray(Wo, f32),
         (np.asarray(beta, f32) @ np.asarray(Wo, f32) + np.asarray(bo, f32))[None, :]],
        axis=0,
    ).astype(f32)
    cst = np.zeros((128, CW), f32)
    cst[:, C_ID:C_ID + 128] = np.eye(128, dtype=f32)
    cst[:, C_BV:C_BV + 512] = np.tile(np.asarray(bv, f32), 16)
    cst[:, C_BQ] = np.tile(np.asarray(bq, f32), 4)
    cst[:, C_BK] = np.tile(np.asarray(bk, f32), 4)
    cst[:, C_EPS] = LN_EPS
    cst[0:D + 1, C_WOA:C_WOA + 128] = woa
    cst[:, C_WQ:C_WQ + 128] = np.tile(np.asarray(Wq, f32), (1, 4))
    wkv = np.concatenate(
        [np.asarray(Wk, f32), np.asarray(Wv, f32)], axis=1
    ).astype(bf16)
    common = {"x2b": x2b, "cst": cst, "wkv": np.ascontiguousarray(wkv)}
    return [
        dict(common, x1=input1[c * MSH:(c + 1) * MSH]) for c in range(NCORES)
    ]


def kernel(input1, input2, Wq, bq, Wk, bk, Wv, bv, gamma, beta, Wo, bo):
    _import_concourse()
    from concourse.bass_utils import run_bass_kernel_spmd

    nc = build()
    in_maps = host_inputs(
        input1, input2, Wq, bq, Wk, bk, Wv, bv, gamma, beta, Wo, bo
    )
    res = run_bass_kernel_spmd(nc, in_maps, list(range(NCORES)))
    return np.concatenate(
        [np.asarray(res.results[c]["out"]) for c in range(NCORES)], axis=0
    ).astype(np.float32)


# revision 10
# speedup vs baseline: 1.4105x; 1.0496x over previous
"""Trainium2 Bass kernel for fused attention + LayerNorm + projection.

Computation (per reference):
    q = input1 @ Wq + bq                       [8192, 32]
    k = input2 @ Wk + bk                       [8192, 32]
    v = input2 @ Wv + bv                       [8192, 32]
    P = softmax(q @ k.T, axis=-1)              [8192, 8192]
    fused = P @ v                              [8192, 32]
    out = LayerNorm(fused) * gamma + beta @ Wo + bo   [8192, 128]

Sharding: data-parallel over rows of input1 (1024 rows per core, 8 cores);
input2 and weights replicated.

Algebraic simplifications (validated vs reference, rel err ~6e-3):
  - softmax normalization (and max-subtraction) skipped: LayerNorm is
    invariant to a positive per-row scale, so exp(s) @ v is normalized for
    free by LN.
  - gamma folded into Wo (diag(gamma) @ Wo), beta/bo folded into an extra
    contraction row via an augmented ones-row, on the host.

Dataflow per core (ACT-exp is the roofline: 8.4M exps at 1 elem/lane/cyc
@1.2GHz ~= 64us; everything else is arranged to overlap under it):
  - x2 is cast to bf16 on host and DMA'd HBM->SBUF through the XBAR
    transpose unit: x2T [128d, 8192n] lands directly with the contraction
    dim on partitions - zero PE transposes for the k/v projections.
  - prep is batched 4 groups (16 chunks) per dedicated 2-bank PSUM tile:
    4x 4-way column-tiled k quads into bank0 (kT banded into kstack f32r,
    chunk c at partitions 32*(c%4)) and 16 v matmuls into bank1 (v
    natural [n,32] -> vstack bf16), then ONE bias-add each. All 4 batches
    are emitted up front so prep races ahead of the attention stream
    during its PE-idle ramp.
  - q: x1 stays f32 for precision; PE-transposed (bank-alternated), then
    a 4-way col-tiled f32r matmul replicates qT into all 4 partition
    bands of qt_rep (fp32 matmuls would split into HI/LO pairs and run
    4x slower - everything on the PE is f32r or bf16).
  - scores: per (pass, group) ONE 4-way row-tiled f32r matmul quad
    (tile_position=(32j,0), concurrent on the PE sub-arrays) writes 4
    PSUM banks = scoresT for 4 chunks x 512 m.
  - exp on ACT straight out of PSUM, 2x [128,1024] bf16 -> pt. Emission
    is software-pipelined (scores g | exp g | AV g-1) so the PE finishes
    each score quad inside the previous exp's shadow and ACT never gaps.
  - AV: 4-way col-tiled bf16 quad accumulates fusedT into one PSUM bank
    across all 16 groups (start on g==0, stop on g==15).
  - m is processed in 2 sequential passes of 512 rows; pass 0's LayerNorm
    + output projection overlaps pass 1's attention stream.
  - PSUM: 2x [128,1024] score tiles (4 banks) + prep tile (2 banks) +
    1 AV bank + 1 LN bank = 8.
  - LayerNorm (batched, 4 blocks per pass): bn_stats/bn_aggr; rstd is
    computed on the DVE with the bit-trick rsqrt + 2 Newton steps so the
    ACT engine NEVER leaves the exp table set (a Ln/Sqrt activation would
    cost two ~2.7us ACT_TABLE_LOAD+DRAIN stalls per pass); projection
    through an augmented [33,512] lhsT with a persistent ones row; one
    rearranged output DMA per pass.
"""

import os
import sys

import numpy as np

N1 = 8192
N2 = 8192
DIN = 128
D = 32
DOUT = 128
NCORES = 8
MSH = N1 // NCORES          # rows per core
NCH = N2 // 128             # 64 in2 chunks
NG = NCH // 4               # 16 groups of 4 chunks
NB = NG // 4                # 4 prep batches of 4 groups
NP = MSH // 512             # 2 m-passes of 512 rows
LN_EPS = 1e-5

_CACHE = {}


def _import_concourse():
    try:
        import concourse.bass  # noqa: F401
    except ImportError:
        for p in ("/opt/trn_rl_repo", os.path.expanduser("~/.axon_site/_ro/trn_rl_repo")):
            if os.path.isdir(p) and p not in sys.path:
                sys.path.insert(0, p)


# Packed f32 consts layout (columns of the [128, CW] "cst" tensor).
C_ID = 0          # ident [128,128]
C_BV = 128        # bv16  [128,512]
C_BQ = 640        # bq4   [128,1]
C_BK = 641        # bk4   [128,1]
C_EPS = 642       # epsc  [128,1]
C_WOA = 643       # woa   [33,128] (rows 0:33)
C_WQ = 771        # wq4   [128,128] (Wq tiled 4x in cols)
CW = 899


def build():
    """Build (and cache) the compiled single-core SPMD Bass program."""
    if "nc" in _CACHE:
        return _CACHE["nc"]
    _import_concourse()
    import concourse.bacc as bacc
    import concourse.tile as tile
    from concourse import mybir

    f32 = mybir.dt.float32
    f32r = mybir.dt.float32r
    i32 = mybir.dt.int32
    bf16 = mybir.dt.bfloat16
    AF = mybir.ActivationFunctionType
    OP = mybir.AluOpType

    nc = bacc.Bacc(None, target_bir_lowering=False, debug=False)

    x1 = nc.dram_tensor("x1", [MSH, DIN], f32, kind="ExternalInput")
    x2b = nc.dram_tensor("x2b", [N2, DIN], bf16, kind="ExternalInput")
    cst_d = nc.dram_tensor("cst", [128, CW], f32, kind="ExternalInput")
    wkv_d = nc.dram_tensor("wkv", [DIN, 2 * D], bf16, kind="ExternalInput")
    out_d = nc.dram_tensor("out", [MSH, DOUT], f32, kind="ExternalOutput")

    from contextlib import ExitStack

    with tile.TileContext(nc) as tc, ExitStack() as outer:
        consts = outer.enter_context(tc.tile_pool(name="consts", bufs=1))
        persist = outer.enter_context(tc.tile_pool(name="persist", bufs=1))

        cst = consts.tile([128, CW], f32)
        wkv = consts.tile([DIN, 2 * D], bf16)
        ident = cst[:, C_ID:C_ID + 128]
        bv16 = cst[:, C_BV:C_BV + 512]
        bq4 = cst[:, C_BQ:C_BQ + 1]
        bk4 = cst[:, C_BK:C_BK + 1]
        epsc = cst[:, C_EPS:C_EPS + 1]
        wk = wkv[:, 0:D]
        wv = wkv[:, D:2 * D]

        woa_r = consts.tile([D + 1, DOUT], f32r)
        nc.vector.tensor_copy(woa_r, cst[0:D + 1, C_WOA:C_WOA + 128])
        wq4_r = consts.tile([DIN, 128], f32r)
        nc.vector.tensor_copy(wq4_r, cst[:, C_WQ:C_WQ + 128])
        magic = consts.tile([128, NP * 4], i32)
        nc.vector.memset(magic, 0x5F3759DF)

        # Pull the exp table load (~2.7us) into the initial DMA window.
        warm = consts.tile([1, 8], f32)
        nc.scalar.activation(warm, cst[0:1, 0:8], AF.Exp)

        x2t = persist.tile([128, N2], bf16)             # x2 transposed (d on partitions)
        kstack = persist.tile([128, NG * 128], f32r)    # kT chunk c: [32*(c%4):+32, 128*(c//4):+128]
        vstack = persist.tile([128, NCH * D], bf16)     # v chunk c: [:, 32*c:+32]
        qt_rep = persist.tile([128, MSH], f32r)         # qT replicated in 4 bands
        x1t_all = persist.tile([128, MSH], f32r)        # input1 shard transposed
        fusedT = persist.tile([D, MSH], f32)            # v.T @ P.T after band reduce
        na_all = persist.tile([D + 1, 512], f32r)       # augmented LN lhsT; row D = ones

        with (
            tc.tile_pool(name="sc_ps", bufs=2, space="PSUM") as sc_ps,
            tc.tile_pool(name="kv_ps", bufs=1, space="PSUM") as kv_ps,
            tc.tile_pool(name="av_ps", bufs=1, space="PSUM") as av_ps,
            tc.tile_pool(name="pp_ps", bufs=1, space="PSUM") as pp_ps,
            tc.tile_pool(name="x1load", bufs=1) as x1load,
            tc.tile_pool(name="pt", bufs=4) as ptp,
            tc.tile_pool(name="tmp32", bufs=2) as tmp32p,
            tc.tile_pool(name="fb", bufs=2) as fbp,
            tc.tile_pool(name="cent", bufs=2) as centp,
            tc.tile_pool(name="stat", bufs=2) as statp,
            tc.tile_pool(name="lnagg", bufs=8) as lnagg,
            tc.tile_pool(name="outsb", bufs=2) as outsbp,
        ):
            nc.vector.tensor_scalar(
                na_all[D:D + 1, :], bv16[0:1, :], 0.0, 1.0,
                op0=OP.mult, op1=OP.add,
            )

            # ---- input DMAs: the q-prep chain (cst for ident/wq4, then x1)
            # gates the stream start, so those go down the queue first.
            nc.sync.dma_start(out=cst, in_=cst_d[:])
            x1_sb = x1load.tile([128, MSH // 128, 128], f32)
            nc.sync.dma_start(
                out=x1_sb, in_=x1[:].rearrange("(t p) d -> p t d", p=128)
            )
            nc.sync.dma_start(out=wkv, in_=wkv_d[:])
            for g in range(NG):
                nc.sync.dma_start(
                    out=x2t[:, g * 512:(g + 1) * 512],
                    in_=x2b[g * 512:(g + 1) * 512, :],
                    transpose=True,
                )

            # ---- q prep (through sc tiles; transposes alternate banks) ----
            qprio = tc.high_priority()
            qprio.__enter__()
            tsp = sc_ps.tile([128, 1024], f32, tag="sc")
            for t in range(MSH // 128):
                col = (t % 2) * 512 + (t // 2) * 128
                nc.tensor.transpose(tsp[:, col:col + 128], x1_sb[:, t, :], ident)
            for h in range(2):
                nc.vector.tensor_copy(
                    x1t_all[:].rearrange("p (t d) -> p t d", d=128)[:, h::2, :],
                    tsp[:, h * 512:(h + 1) * 512].rearrange(
                        "p (t d) -> p t d", d=128
                    ),
                )
            qsp = sc_ps.tile([128, 1024], f32, tag="sc")
            for t2 in range(MSH // 512):
                nc.tensor.matmul(
                    qsp[:, t2 * 512:(t2 + 1) * 512],
                    lhsT=wq4_r,
                    rhs=x1t_all[:, t2 * 512:(t2 + 1) * 512],
                    start=True,
                    stop=True,
                )
                nc.vector.tensor_scalar_add(
                    qt_rep[:, t2 * 512:(t2 + 1) * 512],
                    qsp[:, t2 * 512:(t2 + 1) * 512],
                    bq4,
                )
            qprio.__exit__(None, None, None)

            # ---- k/v prep: 4 batches of 4 groups, all emitted up front ----
            def prep_batch(B):
                pp = kv_ps.tile([128, 1024], f32, tag="kv")
                for gi in range(4):
                    g = 4 * B + gi
                    for j in range(4):
                        nc.tensor.matmul(
                            pp[32 * j:32 * (j + 1), gi * 128:(gi + 1) * 128],
                            lhsT=wk,
                            rhs=x2t[:, (4 * g + j) * 128:(4 * g + j + 1) * 128],
                            start=True,
                            stop=True,
                            tile_position=(0, 32 * j),
                        )
                    nc.vector.tensor_scalar_add(
                        kstack[:, g * 128:(g + 1) * 128],
                        pp[:, gi * 128:(gi + 1) * 128], bk4
                    )
                for ci in range(16):
                    c = 16 * B + ci
                    nc.tensor.matmul(
                        pp[:, 512 + 32 * ci:512 + 32 * (ci + 1)],
                        lhsT=x2t[:, c * 128:(c + 1) * 128],
                        rhs=wv,
                        start=True,
                        stop=True,
                    )
                nc.vector.tensor_add(
                    vstack[:, B * 512:(B + 1) * 512], pp[:, 512:1024], bv16
                )

            for B in range(NB):
                prep_batch(B)

            # ---- attention stream pieces ----
            def score_exp(p, g):
                m0 = p * 512
                ta = sc_ps.tile([128, 1024], f32, tag="sc")
                tb = sc_ps.tile([128, 1024], f32, tag="sc")
                for j in range(4):
                    t_ = ta if j < 2 else tb
                    c0 = 512 * (j % 2)
                    nc.tensor.matmul(
                        t_[:, c0:c0 + 512],
                        lhsT=kstack[32 * j:32 * (j + 1), g * 128:(g + 1) * 128],
                        rhs=qt_rep[32 * j:32 * (j + 1), m0:m0 + 512],
                        start=True,
                        stop=True,
                        tile_position=(32 * j, 0),
                    )
                pta = ptp.tile([128, 1024], bf16, tag="pt")
                nc.scalar.activation(pta, ta, AF.Exp)
                ptb = ptp.tile([128, 1024], bf16, tag="pt")
                nc.scalar.activation(ptb, tb, AF.Exp)
                return pta, ptb

            def av_quad(g, pts, av_acc):
                pta, ptb = pts
                for j in range(4):
                    c = 4 * g + j
                    pt_ = pta if j < 2 else ptb
                    c0 = 512 * (j % 2)
                    nc.tensor.matmul(
                        av_acc[32 * j:32 * (j + 1), :],
                        lhsT=vstack[:, D * c:D * (c + 1)],
                        rhs=pt_[:, c0:c0 + 512],
                        start=(g == 0),
                        stop=(g == NG - 1),
                        tile_position=(0, 32 * j),
                        skip_group_check=True,
                    )

            def band_reduce(p, av_acc):
                m0 = p * 512
                t1 = tmp32p.tile([D, 512], f32, tag="t1")
                nc.vector.tensor_copy(t1, av_acc[0:32, :])
                t2 = tmp32p.tile([D, 512], f32, tag="t2")
                nc.vector.tensor_add(t2, t1, av_acc[32:64, :])
                nc.vector.tensor_add(t1, t2, av_acc[64:96, :])
                nc.vector.tensor_add(
                    fusedT[:, m0:m0 + 512], t1, av_acc[96:128, :]
                )

            def rsqrt_dve(rstd, ve, p):
                """rstd = ve**-0.5 on the DVE (bit-trick seed + 2 Newton)."""
                sh = lnagg.tile([128, 4], i32, tag="sh")
                nc.vector.tensor_scalar(
                    sh, ve.bitcast(i32), 1, None, op0=OP.logical_shift_right
                )
                s_i = lnagg.tile([128, 4], i32, tag="si")
                nc.vector.tensor_tensor(
                    s_i, magic[:, 4 * p:4 * (p + 1)], sh, op=OP.subtract
                )
                cur = s_i.bitcast(f32)
                for it in range(1):
                    a = lnagg.tile([128, 4], f32, tag=f"a{it}")
                    nc.vector.tensor_tensor(a, cur, cur, op=OP.mult)
                    nc.vector.tensor_tensor(a, a, ve, op=OP.mult)
                    nc.vector.tensor_scalar(
                        a, a, -0.5, 1.5, op0=OP.mult, op1=OP.add
                    )
                    nxt = rstd if it == 0 else lnagg.tile([128, 4], f32, tag="s1")
                    nc.vector.tensor_tensor(nxt, cur, a, op=OP.mult)
                    cur = nxt

            # ---- LayerNorm + output projection for one 512-row pass ----
            def ln_pass(p, psp, ptag):
                m0 = p * 512
                fp1 = psp.tile([128, 512], f32, tag=ptag)
                for bi in range(4):
                    nc.tensor.transpose(
                        fp1[:, 32 * bi:32 * (bi + 1)],
                        fusedT[:, m0 + bi * 128:m0 + (bi + 1) * 128],
                        ident[0:D, 0:D],
                    )
                fsb = fbp.tile([128, 128], f32, tag="f")
                nc.vector.tensor_copy(fsb, fp1[:, 0:128])
                mv = lnagg.tile([128, 4, 2], f32, tag="mv")
                for bi in range(4):
                    st = statp.tile([128, 6], f32, tag="st")
                    nc.vector.bn_stats(out=st, in_=fsb[:, 32 * bi:32 * (bi + 1)])
                    nc.vector.bn_aggr(out=mv[:, bi, :], in_=st)
                ve = lnagg.tile([128, 4], f32, tag="ve")
                nc.vector.tensor_scalar_add(ve, mv[:, :, 1], epsc)
                rstd = lnagg.tile([128, 4], f32, tag="rs")
                rsqrt_dve(rstd, ve, p)
                cent = centp.tile([128, 128], f32, tag="c")
                for bi in range(4):
                    nc.vector.tensor_scalar(
                        cent[:, 32 * bi:32 * (bi + 1)],
                        fsb[:, 32 * bi:32 * (bi + 1)],
                        mv[:, bi, 0:1], rstd[:, bi:bi + 1],
                        op0=OP.subtract, op1=OP.mult,
                    )
                fp2 = psp.tile([128, 512], f32, tag=ptag)
                for bi in range(4):
                    nc.tensor.transpose(
                        fp2[0:D, 128 * bi:128 * (bi + 1)],
                        cent[:, 32 * bi:32 * (bi + 1)],
                        ident,
                    )
                nc.vector.tensor_copy(na_all[0:D, :], fp2[0:D, 0:512])
                fp3 = psp.tile([128, 512], f32, tag=ptag)
                for bi in range(4):
                    nc.tensor.matmul(
                        fp3[:, 128 * bi:128 * (bi + 1)],
                        lhsT=na_all[:, 128 * bi:128 * (bi + 1)],
                        rhs=woa_r,
                        start=True,
                        stop=True,
                    )
                osb = outsbp.tile([128, 4, DOUT], f32, tag="o")
                nc.vector.tensor_copy(osb, fp3[:].rearrange("p (t d) -> p t d", t=4))
                nc.sync.dma_start(
                    out=out_d[m0:m0 + 512, :].rearrange("(t p) d -> p t d", p=128),
                    in_=osb,
                )

            # ---- main: two sequential m-passes, software-pipelined AV ----
            av_accs = [None, None]
            for p in range(NP):
                av_acc = av_ps.tile([128, 512], f32, tag="av", name=f"av{p}")
                av_accs[p] = av_acc
                pts = None
                for g in range(NG):
                    new_pts = score_exp(p, g)
                    if pts is not None:
                        av_quad(g - 1, pts, av_acc)
                    pts = new_pts
                    if p == 1 and g == 2:
                        ln_pass(0, pp_ps, "pp")
                av_quad(NG - 1, pts, av_acc)
                band_reduce(p, av_acc)
            ln_pass(1, sc_ps, "sc")

    nc.compile()
    _CACHE["nc"] = nc
    return nc


def host_inputs(input1, input2, Wq, bq, Wk, bk, Wv, bv, gamma, beta, Wo, bo):
    """Per-core input maps (host-side weight folding + bf16 casts)."""
    import ml_dtypes
    f32 = np.float32
    bf16 = ml_dtypes.bfloat16
    input1 = np.ascontiguousarray(np.asarray(input1, f32))
    x2b = np.ascontiguousarray(np.asarray(input2, f32).astype(bf16))
    woa = np.concatenate(
        [np.asarray(gamma, f32)[:, None] * np.asarray(Wo, f32),
         (np.asarray(beta, f32) @ np.asarray(Wo, f32) + np.asarray(bo, f32))[None, :]],
        axis=0,
    ).astype(f32)
    cst = np.zeros((128, CW), f32)
    cst[:, C_ID:C_ID + 128] = np.eye(128, dtype=f32)
    cst[:, C_BV:C_BV + 512] = np.tile(np.asarray(bv, f32), 16)
    cst[:, C_BQ] = np.tile(np.asarray(bq, f32), 4)
    cst[:, C_BK] = np.tile(np.asarray(bk, f32), 4)
    cst[:, C_EPS] = LN_EPS
    cst[0:D + 1, C_WOA:C_WOA + 128] = woa
    cst[:, C_WQ:C_WQ + 128] = np.tile(np.asarray(Wq, f32), (1, 4))
    wkv = np.concatenate(
        [np.asarray(Wk, f32), np.asarray(Wv, f32)], axis=1
    ).astype(bf16)
    common = {"x2b": x2b, "cst": cst, "wkv": np.ascontiguousarray(wkv)}
    return [
        dict(common, x1=input1[c * MSH:(c + 1) * MSH]) for c in range(NCORES)
    ]


def kernel(input1, input2, Wq, bq, Wk, bk, Wv, bv, gamma, beta, Wo, bo):
    _import_concourse()
    from concourse.bass_utils import run_bass_kernel_spmd

    nc = build()
    in_maps = host_inputs(
        input1, input2, Wq, bq, Wk, bk, Wv, bv, gamma, beta, Wo, bo
    )
    res = run_bass_kernel_spmd(nc, in_maps, list(range(NCORES)))
    return np.concatenate(
        [np.asarray(res.results[c]["out"]) for c in range(NCORES)], axis=0
    ).astype(np.float32)
